# revision 3
# baseline (speedup 1.0000x reference)
"""7x7 valid conv2d on [8192, 8192] fp32, distributed over 8 NeuronCores.

Strategy: row-shard the image across 8 cores (host-side overlapping slices
provide the 6-row halo). On each core the convolution runs on the tensor
engine as banded-Toeplitz matmuls: for kernel column j, a stationary matrix
B_j[p, m] = weight[p - m, j] turns a matmul over 128 input rows into a 7-tap
convolution along H producing 122 output rows; the 7 kernel columns
accumulate in PSUM using column-shifted rhs windows. Bias is folded into the
PSUM->SBUF copy as an immediate.
"""

import numpy as np

KH = KW = 7
H = W = 8192
OH = H - KH + 1  # 8186
OW = W - KW + 1
NCORES = 8
P = 128
M_FULL = P - (KH - 1)  # 122 output rows per row-tile
N_TILE = 512

BAND = 1024              # output rows per core (last core overlaps)
B_IN = BAND + KH - 1     # 1030 input rows per core

MM_DTYPE = "float32r"    # matmul operand dtype: float32r | float32


def _build_program(bias_val, band_out, w_in):
    import concourse.bacc as bacc
    import concourse.mybir as mybir
    import concourse.tile as tile

    mm_dt = getattr(mybir.dt, MM_DTYPE)
    f32 = mybir.dt.float32
    w_out = w_in - KW + 1

    nc = bacc.Bacc(
        "TRN2",
        target_bir_lowering=False,
        debug=False,
        enable_asserts=False,
        num_devices=NCORES,
    )

    x_dram = nc.dram_tensor("x", [band_out + KH - 1, w_in], mm_dt, kind="ExternalInput")
    wb_dram = nc.dram_tensor("wband", [P, KW, M_FULL], mm_dt, kind="ExternalInput")
    out_dram = nc.dram_tensor("out", [band_out, w_out], f32, kind="ExternalOutput")

    n_row_tiles = (band_out + M_FULL - 1) // M_FULL

    with tile.TileContext(nc) as tc:
        with (
            tc.tile_pool(name="const", bufs=1) as cpool,
            tc.tile_pool(name="xp", bufs=2) as xpool,
            tc.tile_pool(name="op", bufs=4) as opool,
            tc.tile_pool(name="pp", bufs=8, space="PSUM") as pspool,
        ):
            w_sb = cpool.tile([P, KW, M_FULL], mm_dt)
            nc.sync.dma_start(w_sb[:], wb_dram.ap()[:])

            for t in range(n_row_tiles):
                r0 = t * M_FULL
                m = min(M_FULL, band_out - r0)
                k = m + KH - 1
                x_sb = xpool.tile([P, w_in], mm_dt, tag="x")
                nc.sync.dma_start(x_sb[:k, :], x_dram.ap()[r0 : r0 + k, :])
                for c0 in range(0, w_out, N_TILE):
                    wn = min(N_TILE, w_out - c0)
                    ps = pspool.tile([M_FULL, N_TILE], f32, tag="ps")
                    for j in range(KW):
                        nc.tensor.matmul(
                            ps[:m, :wn],
                            w_sb[:k, j, :m],
                            x_sb[:k, c0 + j : c0 + j + wn],
                            start=(j == 0),
                            stop=(j == KW - 1),
                        )
                    o_sb = opool.tile([M_FULL, N_TILE], f32, tag="o")
                    nc.vector.tensor_scalar_add(o_sb[:m, :wn], ps[:m, :wn], bias_val)
                    nc.sync.dma_start(out_dram.ap()[r0 : r0 + m, c0 : c0 + wn], o_sb[:m, :wn])

    nc.compile()
    return nc


def _make_wband(weight):
    wband = np.zeros((P, KW, M_FULL), np.float32)
    idx = np.arange(M_FULL)
    for j in range(KW):
        for d in range(KH):
            wband[idx + d, j, idx] = weight[d, j]
    return wband


class Runner:
    """Compiles the per-core program once and exposes repeatable execution
    on all cores via PJRT (the axon path of run_bass_kernel_spmd, inlined so
    inputs can stay device-resident and calls can be timed)."""

    def __init__(self, bias_val, band_out=BAND, w_in=W, n_cores=NCORES):
        import jax
        import concourse.mybir as mybir
        from concourse import bass2jax
        from jax.sharding import Mesh, PartitionSpec
        from jax.experimental.shard_map import shard_map

        self.n_cores = n_cores
        nc = _build_program(bias_val, band_out, w_in)
        self.nc = nc
        bass2jax.install_neuronx_cc_hook()

        partition_name = (
            nc.partition_id_tensor.name if nc.partition_id_tensor else None
        )
        in_names, out_names, out_avals = [], [], []
        for alloc in nc.m.functions[0].allocations:
            if not isinstance(alloc, mybir.MemoryLocationSet):
                continue
            name = alloc.memorylocations[0].name
            if alloc.kind == "ExternalInput":
                if name != partition_name:
                    in_names.append(name)
            elif alloc.kind == "ExternalOutput":
                out_names.append(name)
                out_avals.append(
                    jax.core.ShapedArray(
                        tuple(alloc.tensor_shape), mybir.dt.np(alloc.dtype)
                    )
                )
        self.in_names, self.out_names, self.out_avals = in_names, out_names, out_avals
        n_params = len(in_names)
        donate = tuple(range(n_params, n_params + len(out_names)))

        def _body(*args):
            operands = list(args)
            if nc.partition_id_tensor is not None:
                operands.append(bass2jax.partition_id_tensor())
            outs = bass2jax._bass_exec_p.bind(
                *operands,
                out_avals=tuple(out_avals),
                in_names=tuple(in_names + out_names)
                + ((nc.partition_id_tensor.name,) if nc.partition_id_tensor else ()),
                out_names=tuple(out_names),
                lowering_input_output_aliases=(),
                sim_require_finite=True,
                sim_require_nnan=True,
                nc=nc,
            )
            return tuple(outs)

        devices = jax.devices()[:n_cores]
        self.mesh = Mesh(np.asarray(devices), ("core",))
        self.pspec = PartitionSpec("core")
        in_specs = (self.pspec,) * (n_params + len(out_names))
        out_specs = (self.pspec,) * len(out_names)
        self.fn = jax.jit(
            shard_map(
                _body,
                mesh=self.mesh,
                in_specs=in_specs,
                out_specs=out_specs,
                check_rep=False,
            ),
            donate_argnums=donate,
            keep_unused=True,
        )

    def put_inputs(self, in_maps):
        """device_put per-core input dicts; returns list of jax arrays."""
        import jax
        from jax.sharding import NamedSharding

        sharding = NamedSharding(self.mesh, self.pspec)
        arrs = []
        for name in self.in_names:
            cat = np.concatenate([np.asarray(m[name]) for m in in_maps], axis=0)
            arrs.append(jax.device_put(cat, sharding))
        return arrs

    def zero_outs(self):
        import jax
        from jax.sharding import NamedSharding

        sharding = NamedSharding(self.mesh, self.pspec)
        return tuple(
            jax.device_put(
                np.zeros((self.n_cores * a.shape[0], *a.shape[1:]), a.dtype), sharding
            )
            for a in self.out_avals
        )

    def run(self, in_arrs, out_bufs):
        """One execution; returns new device output arrays (donates out_bufs)."""
        return self.fn(*in_arrs, *out_bufs)

    def gather(self, outs):
        """Device outputs -> list of per-core dicts of np arrays."""
        res = []
        for c in range(self.n_cores):
            d = {}
            for i, name in enumerate(self.out_names):
                a = self.out_avals[i]
                d[name] = np.asarray(outs[i]).reshape(self.n_cores, *a.shape)[c]
            res.append(d)
        return res


def make_in_maps(x, weight, starts, band_in=B_IN):
    wband = _make_wband(weight)
    return [
        {"x": np.ascontiguousarray(x[s : s + band_in]), "wband": wband}
        for s in starts
    ]


def kernel(x, weight, bias):
    import jax

    x = np.asarray(x, dtype=np.float32)
    weight = np.asarray(weight, dtype=np.float32)
    bias = np.asarray(bias, dtype=np.float32)

    starts = [min(i * BAND, OH - BAND) for i in range(NCORES)]
    runner = Runner(float(bias[0]))
    in_arrs = runner.put_inputs(make_in_maps(x, weight, starts))
    outs = runner.run(in_arrs, runner.zero_outs())
    jax.block_until_ready(outs)
    results = runner.gather(outs)

    out = np.empty((OH, OW), np.float32)
    for s, r in zip(starts, results):
        out[s : s + BAND] = r["out"]
    return out
